# revision 13
# baseline (speedup 1.0000x reference)
"""Trainium2 Bass kernel for MeshDihedralAngleLoss.

Reference computation (per batch b, per edge e with ep = edge_points[b,e] =
[v0, v1, a, b]):
    na = normalize(cross(verts[a]-verts[v0], verts[v1]-verts[v0]))
    nb = normalize(cross(verts[b]-verts[v1], verts[v0]-verts[v1]))
    angle = pi - arccos(clip(dot(na, nb), +-(1-1e-7)))
computed for vert1 and vert2; loss = mean_b mean_e (angle1-angle2)^2.

Algebra: with ea = p2-p0, eb = p1-p0, d3 = p3-p1, nb = cross(d3, -eb) = -m,
    angle = pi - arccos(-u) = arccos(u),  u = dot(na, m)/(|na||m|)
and with q = dot(na, m), w = |na||m|:
    arccos(u) = 2*atan(sqrt((w - q)/(w + q)))
              = 2*atan(exp(0.5*(ln(w - q) - ln(w + q))))     [division-free]
so angle1 - angle2 = 2*(atan(t1) - atan(t2)) and the host applies the *4
factor on the squared sums plus the global mean (the only cross-core step).

Sharding: pure data parallel, core b <- mesh b (B == 8 == n_cores).

Host marshaling: the indexed gather is pure data movement, so it is done
host-side with numpy fancy indexing (same class of marshaling as index
pre-tiling): each core receives its edges' vertex coordinates pre-gathered
(fp16) into the exact plane-major SBUF layout
    pg[t, p, ((m*3 + c)*4 + j)*F + w] = verts_m[ep[e, j], c],
    e = (t*P + p)*F + w
(m = mesh 0/1, c = xyz, j = vertex slot 0..3).  The device then streams
sequential DRAM at full DMA bandwidth -- no per-edge descriptors.

Engine split (software-pipelined by one chunk so DVE never waits):
  DVE:  edge vectors, cross products, dot products (fp16, 2x mode)
  ACT:  squares, sqrt, ln/exp (the division), arctan, final sum-accumulate
  Pool: the f32 tail elementwise glue (pr2, w-+q, clamps, ln-diff, at-diff)
"""

import numpy as np

import concourse.bass as bass
import concourse.mybir as mybir
from concourse.tile import TileContext
from concourse.bass_utils import run_bass_kernel_spmd

dt = mybir.dt
Alu = mybir.AluOpType
AF = mybir.ActivationFunctionType

B, N, E = 8, 100000, 300000
P = 128
F = 586            # edges per partition per chunk
T = 4              # chunks; P*F*T = 300032 >= E (32 zero-padded edges)
EPAD = P * F * T
EPS = 1e-30
EPS2 = 1e-35

_CACHE: dict = {}


def _build_program() -> bass.Bass:
    nc = bass.Bass(trn_type="TRN2")
    f32 = dt.float32
    f16 = dt.float16
    # register the eps consts used as ACT biases (same mechanism as the
    # 0.0/1.0 consts Bass registers at init)
    for cv in (EPS, EPS2):
        eps_t = nc.alloc_sbuf_tensor(f"const-float32-{cv}", [128, 1], f32)
        nc.gpsimd.memset(eps_t.ap(), cv)
        nc.const_aps.aps[(f32, cv)] = eps_t.ap()
    nc.all_engine_barrier()
    pg = nc.dram_tensor("pg", [T, P, 24 * F], f16, kind="ExternalInput")
    out = nc.dram_tensor("out", [P, T], f32, kind="ExternalOutput")

    with TileContext(nc) as tc:
        with (
            tc.tile_pool(name="accp", bufs=1) as accp,
            tc.tile_pool(name="iop", bufs=2) as iop,
            tc.tile_pool(name="wkp", bufs=1) as wkp,
            tc.tile_pool(name="pip", bufs=2) as pip,
            tc.tile_pool(name="smp", bufs=1) as smp,
        ):
            acc = accp.tile([P, T], f32)

            def geo(t):
                """DVE geometry for chunk t + ACT squares; returns (q01, namm)."""
                gall = iop.tile([P, 24 * F], f16, tag="gall")
                nc.sync.dma_start(out=gall[:], in_=pg[t, :, :])
                # planes: [m:2][c:3][j:4][w:F]
                gv = gall[:].rearrange("p (m c j w) -> p m c j w", m=2, c=3, j=4)

                # Edge vectors, both meshes at once, into 5-plane buffers
                # (x,y,z,x,y per mesh) so cross-product component rotation
                # becomes a plain +F / +2F offset shift.
                ea5 = wkp.tile([P, 10 * F], f16, tag="ea5")
                eb5 = wkp.tile([P, 10 * F], f16, tag="eb5")
                d35 = wkp.tile([P, 10 * F], f16, tag="d35")
                for buf, js, jb in ((ea5, 2, 0), (eb5, 1, 0), (d35, 3, 1)):
                    bv = buf[:].rearrange("p (m k w) -> p m k w", m=2, k=5)
                    nc.vector.tensor_tensor(
                        out=bv[:, :, 0:3, :],
                        in0=gv[:, :, :, js, :],
                        in1=gv[:, :, :, jb, :],
                        op=Alu.subtract,
                    )
                    nc.vector.tensor_tensor(
                        out=bv[:, :, 3:5, :],
                        in0=gv[:, :, 0:2, js, :],
                        in1=gv[:, :, 0:2, jb, :],
                        op=Alu.subtract,
                    )

                # na = ea x eb ; mm = d3 x eb  (plane-major, per mesh)
                # packed as namm = [na_m0 | na_m1 | mm_m0 | mm_m1] x 3F
                namm = pip.tile([P, 12 * F], f16, tag="namm")
                nav = namm[:, 0 : 6 * F].rearrange("p (m k w) -> p m k w", m=2, k=3)
                mmv = namm[:, 6 * F : 12 * F].rearrange("p (m k w) -> p m k w", m=2, k=3)
                tmp6 = wkp.tile([P, 6 * F], f16, tag="t6")
                t6v = tmp6[:].rearrange("p (m k w) -> p m k w", m=2, k=3)
                for dstv, av in ((nav, ea5), (mmv, d35)):
                    a5 = av[:].rearrange("p (m k w) -> p m k w", m=2, k=5)
                    b5 = eb5[:].rearrange("p (m k w) -> p m k w", m=2, k=5)
                    nc.vector.tensor_tensor(
                        out=dstv, in0=a5[:, :, 1:4, :], in1=b5[:, :, 2:5, :],
                        op=Alu.mult,
                    )
                    nc.vector.tensor_tensor(
                        out=t6v, in0=a5[:, :, 2:5, :], in1=b5[:, :, 1:4, :],
                        op=Alu.mult,
                    )
                    nc.vector.tensor_tensor(
                        out=dstv, in0=dstv, in1=t6v, op=Alu.subtract
                    )

                # q = dot(na, mm) per edge -> [P, 2F] (m-major)
                prod = wkp.tile([P, 6 * F], f16, tag="prod")
                nc.vector.tensor_tensor(
                    out=prod[:], in0=namm[:, 0 : 6 * F], in1=namm[:, 6 * F : 12 * F],
                    op=Alu.mult,
                )
                pv = prod[:].rearrange("p (m k w) -> p m k w", m=2, k=3)
                q01 = pip.tile([P, 2 * F], f16, tag="q01")
                qv = q01[:].rearrange("p (m w) -> p m w", m=2)
                nc.vector.tensor_tensor(
                    out=qv, in0=pv[:, :, 0, :], in1=pv[:, :, 1, :], op=Alu.add
                )
                nc.vector.tensor_tensor(
                    out=qv, in0=qv, in1=pv[:, :, 2, :], op=Alu.add
                )

                # squares in-place on ACT (runs while DVE streams next chunk)
                nc.scalar.activation(out=namm[:], in_=namm[:], func=AF.Square)
                return q01, namm

            def tail(t, q01, namm):
                """nm2 on DVE; f32 glue on Pool; transcendentals on ACT."""
                sqv = namm[:].rearrange("p (g k w) -> p g k w", g=4, k=3)
                nm2 = pip.tile([P, 4 * F], f16, tag="nm2")
                nmv = nm2[:].rearrange("p (g w) -> p g w", g=4)
                nc.vector.tensor_tensor(
                    out=nmv, in0=sqv[:, :, 0, :], in1=sqv[:, :, 1, :], op=Alu.add
                )
                nc.vector.tensor_tensor(
                    out=nmv, in0=nmv, in1=sqv[:, :, 2, :], op=Alu.add
                )

                # w = sqrt(na2 * m2 + eps); eps keeps padded/degenerate edges
                # finite (q = 0 there -> t = 1 -> both meshes pi/4 -> delta 0)
                pr2 = smp.tile([P, 2 * F], f32, tag="pr2")
                nc.gpsimd.tensor_tensor(
                    out=pr2[:], in0=nm2[:, 0 : 2 * F], in1=nm2[:, 2 * F : 4 * F],
                    op=Alu.mult,
                )
                w01 = smp.tile([P, 2 * F], f32, tag="w01")
                nc.scalar.activation(out=w01[:], in_=pr2[:], func=AF.Sqrt, bias=EPS)

                # t = sqrt((w - q)/(w + q)) = exp(0.5*(ln(w-q) - ln(w+q))).
                # The Ln bias bounds t for the measure-zero edges the reference
                # clips; max(0) clamps guard f32 rounding pushing w -+ q < 0.
                aa = smp.tile([P, 2 * F], f32, tag="aa")
                nc.gpsimd.tensor_tensor(out=aa[:], in0=w01[:], in1=q01[:], op=Alu.subtract)
                nc.gpsimd.tensor_scalar(
                    out=aa[:], in0=aa[:], scalar1=0.0, scalar2=None, op0=Alu.max
                )
                bb = smp.tile([P, 2 * F], f32, tag="bb")
                nc.gpsimd.tensor_tensor(out=bb[:], in0=w01[:], in1=q01[:], op=Alu.add)
                nc.gpsimd.tensor_scalar(
                    out=bb[:], in0=bb[:], scalar1=0.0, scalar2=None, op0=Alu.max
                )
                la = smp.tile([P, 2 * F], f32, tag="la")
                nc.scalar.activation(out=la[:], in_=aa[:], func=AF.Ln, bias=EPS2)
                lb = smp.tile([P, 2 * F], f32, tag="lb")
                nc.scalar.activation(out=lb[:], in_=bb[:], func=AF.Ln, bias=EPS2)
                zv = smp.tile([P, 2 * F], f32, tag="zv")
                nc.gpsimd.tensor_tensor(out=zv[:], in0=la[:], in1=lb[:], op=Alu.subtract)
                tv = smp.tile([P, 2 * F], f32, tag="tv")
                nc.scalar.activation(out=tv[:], in_=zv[:], func=AF.Exp, scale=0.5)
                at = smp.tile([P, 2 * F], f32, tag="at")
                nc.scalar.activation(out=at[:], in_=tv[:], func=AF.Arctan)

                # delta = at0 - at1; acc[:, t] = sum_w delta^2 via ACT accum
                d = smp.tile([P, F], f32, tag="d")
                nc.gpsimd.tensor_tensor(
                    out=d[:], in0=at[:, 0:F], in1=at[:, F : 2 * F], op=Alu.subtract
                )
                dd = smp.tile([P, F], f32, tag="dd")
                nc.scalar.activation(
                    out=dd[:], in_=d[:], func=AF.Square,
                    accum_out=acc[:, t : t + 1],
                )

            pend = None
            for t in range(T):
                cur = geo(t)
                if pend is not None:
                    tail(pend[0], *pend[1])
                pend = (t, cur)
            tail(pend[0], *pend[1])

            nc.sync.dma_start(out=out[:, :], in_=acc[:])

    _split_multi_waits(nc)
    return nc


def _split_multi_waits(nc: bass.Bass) -> None:
    """Walrus accepts at most ONE sync wait per (non-drain) instruction;
    hoist extras onto injected same-engine event-semaphore instructions
    placed immediately before -- semantically identical."""
    import bass_rust

    ctr = 0
    for fn in nc.m.functions:
        for bb in fn.blocks:
            new_list = []
            for inst in bb.instructions:
                si = getattr(inst, "sync_info", None)
                if si is not None and len(si.on_wait) > 1:
                    waits = list(si.on_wait)
                    for w in waits[:-1]:
                        ev = mybir.InstEventSemaphore(name=f"I-waitsplit-{ctr}")
                        ctr += 1
                        ev.engine = inst.engine
                        ev.sync_info = bass_rust.SyncInfo(
                            on_wait=[w], on_update=[]
                        )
                        new_list.append(ev)
                    inst.sync_info = bass_rust.SyncInfo(
                        on_wait=[waits[-1]], on_update=list(si.on_update)
                    )
                new_list.append(inst)
            bb.instructions = new_list


def _get_nc() -> bass.Bass:
    if "nc" not in _CACHE:
        _CACHE["nc"] = _build_program()
    return _CACHE["nc"]


def _prep_in_maps(vert1, vert2, edge_points):
    in_maps = []
    for b in range(B):
        tbl = np.concatenate(
            [np.asarray(vert1[b], np.float32), np.asarray(vert2[b], np.float32)],
            axis=1,
        )  # [N, 6]
        ep = np.asarray(edge_points[b]).astype(np.int32)  # [E, 4]
        pad = np.zeros((EPAD, 4), np.int32)
        pad[:E] = ep
        # edge (t, p, w) = (t*P + p)*F + w; gather rows then lay out
        # plane-major: pg[t, p, (c, j, w)] with c = 3*mesh + xyz
        g = tbl.astype(np.float16)[pad.reshape(T, P, F, 4)]  # [T, P, F, 4, 6]
        pgb = np.ascontiguousarray(g.transpose(0, 1, 4, 3, 2)).reshape(T, P, 24 * F)
        in_maps.append({"pg": pgb})
    return in_maps


def _run(in_maps, **kwargs):
    nc = _get_nc()
    return run_bass_kernel_spmd(nc, in_maps, core_ids=list(range(B)), **kwargs)


def _finalize(results) -> np.float32:
    total = 0.0
    for rmap in results:
        total += float(np.asarray(rmap["out"], np.float64).sum())
    # angle diff = 2*(atan1 - atan2)  ->  factor 4 on the squared sums
    return np.float32(4.0 * total / (B * E))


def kernel(vert1, vert2, edge_points) -> np.ndarray:
    in_maps = _prep_in_maps(vert1, vert2, edge_points)
    res = _run(in_maps)
    return _finalize(res.results)


# revision 14
# speedup vs baseline: 1.7517x; 1.7517x over previous
"""Trainium2 Bass kernel for MeshDihedralAngleLoss.

Reference computation (per batch b, per edge e with ep = edge_points[b,e] =
[v0, v1, a, b]):
    na = normalize(cross(verts[a]-verts[v0], verts[v1]-verts[v0]))
    nb = normalize(cross(verts[b]-verts[v1], verts[v0]-verts[v1]))
    angle = pi - arccos(clip(dot(na, nb), +-(1-1e-7)))
computed for vert1 and vert2; loss = mean_b mean_e (angle1-angle2)^2.

Algebra: with ea = p2-p0, eb = p1-p0, d3 = p3-p1, nb = cross(d3, -eb) = -m,
    angle = pi - arccos(-u) = arccos(u),  u = dot(na, m)/(|na||m|)
and with q = dot(na, m), w = |na||m|:
    arccos(u) = 2*atan(sqrt((w - q)/(w + q)))
              = 2*atan(exp(0.5*(ln(w - q) - ln(w + q))))     [division-free]
so angle1 - angle2 = 2*(atan(t1) - atan(t2)) and the host applies the *4
factor on the squared sums plus the global mean (the only cross-core step).

Sharding: pure data parallel, core b <- mesh b (B == 8 == n_cores).

Host marshaling: the indexed gather is pure data movement, so it is done
host-side with numpy fancy indexing (same class of marshaling as index
pre-tiling): each core receives its edges' vertex coordinates pre-gathered
(fp16) into the exact plane-major SBUF layout
    pg[t, p, ((m*3 + c)*4 + j)*F + w] = verts_m[ep[e, j], c],
    e = (t*P + p)*F + w
(m = mesh 0/1, c = xyz, j = vertex slot 0..3).  The device then streams
sequential DRAM at full DMA bandwidth -- no per-edge descriptors.

Engine split (software-pipelined by one chunk so DVE never waits on ACT):
  DVE:  edge vectors, cross products, dot products (fp16, 2x mode) + f32 glue
  ACT:  squares, sqrt, ln/exp (the division), arctan, final sum-accumulate
"""

import numpy as np

import concourse.bass as bass
import concourse.mybir as mybir
from concourse.tile import TileContext
from concourse.bass_utils import run_bass_kernel_spmd

dt = mybir.dt
Alu = mybir.AluOpType
AF = mybir.ActivationFunctionType

B, N, E = 8, 100000, 300000
P = 128
F = 586            # edges per partition per chunk
T = 4              # chunks; P*F*T = 300032 >= E (32 zero-padded edges)
EPAD = P * F * T
EPS = 1e-30
EPS2 = 1e-35

_CACHE: dict = {}


def _build_program() -> bass.Bass:
    nc = bass.Bass(trn_type="TRN2")
    f32 = dt.float32
    f16 = dt.float16
    # register the eps consts used as ACT biases (same mechanism as the
    # 0.0/1.0 consts Bass registers at init)
    for cv in (EPS, EPS2):
        eps_t = nc.alloc_sbuf_tensor(f"const-float32-{cv}", [128, 1], f32)
        nc.gpsimd.memset(eps_t.ap(), cv)
        nc.const_aps.aps[(f32, cv)] = eps_t.ap()
    nc.all_engine_barrier()
    pg = nc.dram_tensor("pg", [T, P, 24 * F], f16, kind="ExternalInput")
    out = nc.dram_tensor("out", [P, T], f32, kind="ExternalOutput")

    with TileContext(nc) as tc:
        with (
            tc.tile_pool(name="accp", bufs=1) as accp,
            tc.tile_pool(name="iop", bufs=2) as iop,
            tc.tile_pool(name="wkp", bufs=1) as wkp,
            tc.tile_pool(name="pip", bufs=2) as pip,
            tc.tile_pool(name="smp", bufs=1) as smp,
        ):
            acc = accp.tile([P, T], f32)

            def geo(t):
                """DVE geometry for chunk t + ACT squares; returns (q01, namm)."""
                gall = iop.tile([P, 24 * F], f16, tag="gall")
                nc.sync.dma_start(out=gall[:], in_=pg[t, :, :])
                # planes: [m:2][c:3][j:4][w:F]
                gv = gall[:].rearrange("p (m c j w) -> p m c j w", m=2, c=3, j=4)

                # Edge vectors, both meshes at once, into 5-plane buffers
                # (x,y,z,x,y per mesh) so cross-product component rotation
                # becomes a plain +F / +2F offset shift.
                ea5 = wkp.tile([P, 10 * F], f16, tag="ea5")
                eb5 = wkp.tile([P, 10 * F], f16, tag="eb5")
                d35 = wkp.tile([P, 10 * F], f16, tag="d35")
                for buf, js, jb in ((ea5, 2, 0), (eb5, 1, 0), (d35, 3, 1)):
                    bv = buf[:].rearrange("p (m k w) -> p m k w", m=2, k=5)
                    nc.vector.tensor_tensor(
                        out=bv[:, :, 0:3, :],
                        in0=gv[:, :, :, js, :],
                        in1=gv[:, :, :, jb, :],
                        op=Alu.subtract,
                    )
                    nc.vector.tensor_tensor(
                        out=bv[:, :, 3:5, :],
                        in0=gv[:, :, 0:2, js, :],
                        in1=gv[:, :, 0:2, jb, :],
                        op=Alu.subtract,
                    )

                # na = ea x eb ; mm = d3 x eb  (plane-major, per mesh)
                # packed as namm = [na_m0 | na_m1 | mm_m0 | mm_m1] x 3F
                namm = pip.tile([P, 12 * F], f16, tag="namm")
                nav = namm[:, 0 : 6 * F].rearrange("p (m k w) -> p m k w", m=2, k=3)
                mmv = namm[:, 6 * F : 12 * F].rearrange("p (m k w) -> p m k w", m=2, k=3)
                tmp6 = wkp.tile([P, 6 * F], f16, tag="t6")
                t6v = tmp6[:].rearrange("p (m k w) -> p m k w", m=2, k=3)
                for dstv, av in ((nav, ea5), (mmv, d35)):
                    a5 = av[:].rearrange("p (m k w) -> p m k w", m=2, k=5)
                    b5 = eb5[:].rearrange("p (m k w) -> p m k w", m=2, k=5)
                    nc.vector.tensor_tensor(
                        out=dstv, in0=a5[:, :, 1:4, :], in1=b5[:, :, 2:5, :],
                        op=Alu.mult,
                    )
                    nc.vector.tensor_tensor(
                        out=t6v, in0=a5[:, :, 2:5, :], in1=b5[:, :, 1:4, :],
                        op=Alu.mult,
                    )
                    nc.vector.tensor_tensor(
                        out=dstv, in0=dstv, in1=t6v, op=Alu.subtract
                    )

                # q = dot(na, mm) per edge -> [P, 2F] (m-major)
                prod = wkp.tile([P, 6 * F], f16, tag="prod")
                nc.vector.tensor_tensor(
                    out=prod[:], in0=namm[:, 0 : 6 * F], in1=namm[:, 6 * F : 12 * F],
                    op=Alu.mult,
                )
                pv = prod[:].rearrange("p (m k w) -> p m k w", m=2, k=3)
                q01 = pip.tile([P, 2 * F], f16, tag="q01")
                qv = q01[:].rearrange("p (m w) -> p m w", m=2)
                nc.vector.tensor_tensor(
                    out=qv, in0=pv[:, :, 0, :], in1=pv[:, :, 1, :], op=Alu.add
                )
                nc.vector.tensor_tensor(
                    out=qv, in0=qv, in1=pv[:, :, 2, :], op=Alu.add
                )

                # squares in-place on ACT (runs while DVE streams next chunk)
                nc.scalar.activation(out=namm[:], in_=namm[:], func=AF.Square)
                return q01, namm

            def tail(t, q01, namm):
                """Chunk tail, emitted after the NEXT chunk's geometry so the
                DVE never stalls on the ACT results it consumes."""
                sqv = namm[:].rearrange("p (g k w) -> p g k w", g=4, k=3)
                nm2 = pip.tile([P, 4 * F], f16, tag="nm2")
                nmv = nm2[:].rearrange("p (g w) -> p g w", g=4)
                nc.vector.tensor_tensor(
                    out=nmv, in0=sqv[:, :, 0, :], in1=sqv[:, :, 1, :], op=Alu.add
                )
                nc.vector.tensor_tensor(
                    out=nmv, in0=nmv, in1=sqv[:, :, 2, :], op=Alu.add
                )

                # w = sqrt(na2 * m2 + eps); eps keeps padded/degenerate edges
                # finite (q = 0 there -> t = 1 -> both meshes pi/4 -> delta 0)
                pr2 = smp.tile([P, 2 * F], f32, tag="pr2")
                nc.vector.tensor_tensor(
                    out=pr2[:], in0=nm2[:, 0 : 2 * F], in1=nm2[:, 2 * F : 4 * F],
                    op=Alu.mult,
                )
                w01 = smp.tile([P, 2 * F], f32, tag="w01")
                nc.scalar.activation(out=w01[:], in_=pr2[:], func=AF.Sqrt, bias=EPS)

                # t = sqrt((w - q)/(w + q)) = exp(0.5*(ln(w-q) - ln(w+q))).
                # aa|bb packed in one [P, 4F] tile: one clamp, one Ln.  The Ln
                # bias bounds t for the measure-zero edges the reference
                # clips; max(0) clamps guard f32 rounding pushing w -+ q < 0.
                ab = smp.tile([P, 4 * F], f32, tag="ab")
                nc.vector.tensor_tensor(
                    out=ab[:, 0 : 2 * F], in0=w01[:], in1=q01[:], op=Alu.subtract
                )
                nc.vector.tensor_tensor(
                    out=ab[:, 2 * F : 4 * F], in0=w01[:], in1=q01[:], op=Alu.add
                )
                nc.vector.tensor_scalar(
                    out=ab[:], in0=ab[:], scalar1=0.0, scalar2=None, op0=Alu.max
                )
                lab = smp.tile([P, 4 * F], f32, tag="lab")
                nc.scalar.activation(out=lab[:], in_=ab[:], func=AF.Ln, bias=EPS2)
                zv = smp.tile([P, 2 * F], f32, tag="zv")
                nc.vector.tensor_tensor(
                    out=zv[:], in0=lab[:, 0 : 2 * F], in1=lab[:, 2 * F : 4 * F],
                    op=Alu.subtract,
                )
                tv = smp.tile([P, 2 * F], f32, tag="tv")
                nc.scalar.activation(out=tv[:], in_=zv[:], func=AF.Exp, scale=0.5)
                at = smp.tile([P, 2 * F], f32, tag="at")
                nc.scalar.activation(out=at[:], in_=tv[:], func=AF.Arctan)

                # delta = at0 - at1; acc[:, t] = sum_w delta^2 via ACT accum
                d = smp.tile([P, F], f32, tag="d")
                nc.vector.tensor_tensor(
                    out=d[:], in0=at[:, 0:F], in1=at[:, F : 2 * F], op=Alu.subtract
                )
                dd = smp.tile([P, F], f32, tag="dd")
                nc.scalar.activation(
                    out=dd[:], in_=d[:], func=AF.Square,
                    accum_out=acc[:, t : t + 1],
                )

            pend = None
            for t in range(T):
                cur = geo(t)
                if pend is not None:
                    tail(pend[0], *pend[1])
                pend = (t, cur)
            tail(pend[0], *pend[1])

            nc.sync.dma_start(out=out[:, :], in_=acc[:])

    _split_multi_waits(nc)
    return nc


def _split_multi_waits(nc: bass.Bass) -> None:
    """Walrus accepts at most ONE sync wait per (non-drain) instruction;
    hoist extras onto injected same-engine event-semaphore instructions
    placed immediately before -- semantically identical."""
    import bass_rust

    ctr = 0
    for fn in nc.m.functions:
        for bb in fn.blocks:
            new_list = []
            for inst in bb.instructions:
                si = getattr(inst, "sync_info", None)
                if si is not None and len(si.on_wait) > 1:
                    waits = list(si.on_wait)
                    for w in waits[:-1]:
                        ev = mybir.InstEventSemaphore(name=f"I-waitsplit-{ctr}")
                        ctr += 1
                        ev.engine = inst.engine
                        ev.sync_info = bass_rust.SyncInfo(
                            on_wait=[w], on_update=[]
                        )
                        new_list.append(ev)
                    inst.sync_info = bass_rust.SyncInfo(
                        on_wait=[waits[-1]], on_update=list(si.on_update)
                    )
                new_list.append(inst)
            bb.instructions = new_list


def _get_nc() -> bass.Bass:
    if "nc" not in _CACHE:
        _CACHE["nc"] = _build_program()
    return _CACHE["nc"]


def _prep_in_maps(vert1, vert2, edge_points):
    in_maps = []
    for b in range(B):
        tbl = np.concatenate(
            [np.asarray(vert1[b], np.float32), np.asarray(vert2[b], np.float32)],
            axis=1,
        )  # [N, 6]
        ep = np.asarray(edge_points[b]).astype(np.int32)  # [E, 4]
        pad = np.zeros((EPAD, 4), np.int32)
        pad[:E] = ep
        # edge (t, p, w) = (t*P + p)*F + w; gather rows then lay out
        # plane-major: pg[t, p, (c, j, w)] with c = 3*mesh + xyz
        g = tbl.astype(np.float16)[pad.reshape(T, P, F, 4)]  # [T, P, F, 4, 6]
        pgb = np.ascontiguousarray(g.transpose(0, 1, 4, 3, 2)).reshape(T, P, 24 * F)
        in_maps.append({"pg": pgb})
    return in_maps


def _run(in_maps, **kwargs):
    nc = _get_nc()
    return run_bass_kernel_spmd(nc, in_maps, core_ids=list(range(B)), **kwargs)


def _finalize(results) -> np.float32:
    total = 0.0
    for rmap in results:
        total += float(np.asarray(rmap["out"], np.float64).sum())
    # angle diff = 2*(atan1 - atan2)  ->  factor 4 on the squared sums
    return np.float32(4.0 * total / (B * E))


def kernel(vert1, vert2, edge_points) -> np.ndarray:
    in_maps = _prep_in_maps(vert1, vert2, edge_points)
    res = _run(in_maps)
    return _finalize(res.results)


# revision 16
# speedup vs baseline: 1.7956x; 1.0250x over previous
"""Trainium2 Bass kernel for MeshDihedralAngleLoss.

Reference computation (per batch b, per edge e with ep = edge_points[b,e] =
[v0, v1, a, b]):
    na = normalize(cross(verts[a]-verts[v0], verts[v1]-verts[v0]))
    nb = normalize(cross(verts[b]-verts[v1], verts[v0]-verts[v1]))
    angle = pi - arccos(clip(dot(na, nb), +-(1-1e-7)))
computed for vert1 and vert2; loss = mean_b mean_e (angle1-angle2)^2.

Algebra: with ea = p2-p0, eb = p1-p0, d3 = p3-p1, nb = cross(d3, -eb) = -m,
    angle = pi - arccos(-u) = arccos(u),  u = dot(na, m)/(|na||m|)
and with q = dot(na, m), w = |na||m|:
    arccos(u) = 2*atan(sqrt((w - q)/(w + q)))
              = 2*atan(exp(0.5*(ln(w - q) - ln(w + q))))     [division-free]
so angle1 - angle2 = 2*(atan(t1) - atan(t2)) and the host applies the *4
factor on the squared sums plus the global mean (the only cross-core step).

Sharding: pure data parallel, core b <- mesh b (B == 8 == n_cores).

Host marshaling: the indexed gather is pure data movement, so it is done
host-side with numpy fancy indexing (same class of marshaling as index
pre-tiling): each core receives its edges' vertex coordinates pre-gathered
(fp16) into the exact plane-major SBUF layout
    pg[t, p, ((m*3 + c)*4 + j)*F + w] = verts_m[ep[e, j], c],
    e = (t*P + p)*F + w
(m = mesh 0/1, c = xyz, j = vertex slot 0..3).  The device then streams
sequential DRAM at full DMA bandwidth -- no per-edge descriptors.

Engine split (software-pipelined by one chunk so DVE never waits on ACT):
  DVE:  edge vectors, cross products, dot products (fp16, 2x mode) + f32 glue
  ACT:  squares, sqrt, ln/exp (the division), arctan, final sum-accumulate
"""

import numpy as np

import concourse.bass as bass
import concourse.mybir as mybir
from concourse.tile import TileContext
from concourse.bass_utils import run_bass_kernel_spmd

dt = mybir.dt
Alu = mybir.AluOpType
AF = mybir.ActivationFunctionType

B, N, E = 8, 100000, 300000
P = 128
F = 586            # edges per partition per chunk
T = 4              # chunks; P*F*T = 300032 >= E (32 zero-padded edges)
EPAD = P * F * T
EPS = 1e-30
EPS2 = 1e-35

_CACHE: dict = {}


def _build_program() -> bass.Bass:
    nc = bass.Bass(trn_type="TRN2")
    f32 = dt.float32
    f16 = dt.float16
    # register the eps consts used as ACT biases (same mechanism as the
    # 0.0/1.0 consts Bass registers at init)
    for cv in (EPS, EPS2):
        eps_t = nc.alloc_sbuf_tensor(f"const-float32-{cv}", [128, 1], f32)
        nc.gpsimd.memset(eps_t.ap(), cv)
        nc.const_aps.aps[(f32, cv)] = eps_t.ap()
    nc.all_engine_barrier()
    pg = nc.dram_tensor("pg", [T, P, 24 * F], f16, kind="ExternalInput")
    out = nc.dram_tensor("out", [P, T], f32, kind="ExternalOutput")

    with TileContext(nc) as tc:
        with (
            tc.tile_pool(name="accp", bufs=1) as accp,
            tc.tile_pool(name="iop", bufs=2) as iop,
            tc.tile_pool(name="wkp", bufs=1) as wkp,
            tc.tile_pool(name="pip", bufs=2) as pip,
            tc.tile_pool(name="smp", bufs=1) as smp,
        ):
            acc = accp.tile([P, T], f32)

            def geo(t):
                """DVE geometry for chunk t + ACT squares; returns (q01, namm)."""
                gall = iop.tile([P, 24 * F], f16, tag="gall")
                nc.sync.dma_start(out=gall[:], in_=pg[t, :, :])
                # planes: [m:2][c:3][j:4][w:F]
                gv = gall[:].rearrange("p (m c j w) -> p m c j w", m=2, c=3, j=4)

                # Edge vectors, both meshes at once, into 5-plane buffers
                # (x,y,z,x,y per mesh) so cross-product component rotation
                # becomes a plain +F / +2F offset shift.
                ea5 = wkp.tile([P, 10 * F], f16, tag="ea5")
                eb5 = wkp.tile([P, 10 * F], f16, tag="eb5")
                d35 = wkp.tile([P, 10 * F], f16, tag="d35")
                for buf, js, jb in ((ea5, 2, 0), (eb5, 1, 0), (d35, 3, 1)):
                    bv = buf[:].rearrange("p (m k w) -> p m k w", m=2, k=5)
                    nc.vector.tensor_tensor(
                        out=bv[:, :, 0:3, :],
                        in0=gv[:, :, :, js, :],
                        in1=gv[:, :, :, jb, :],
                        op=Alu.subtract,
                    )
                    nc.vector.tensor_tensor(
                        out=bv[:, :, 3:5, :],
                        in0=gv[:, :, 0:2, js, :],
                        in1=gv[:, :, 0:2, jb, :],
                        op=Alu.subtract,
                    )

                # na = ea x eb ; mm = d3 x eb  (plane-major, per mesh)
                # packed as namm = [na_m0 | na_m1 | mm_m0 | mm_m1] x 3F
                namm = pip.tile([P, 12 * F], f16, tag="namm")
                nav = namm[:, 0 : 6 * F].rearrange("p (m k w) -> p m k w", m=2, k=3)
                mmv = namm[:, 6 * F : 12 * F].rearrange("p (m k w) -> p m k w", m=2, k=3)
                tmp6 = wkp.tile([P, 6 * F], f16, tag="t6")
                t6v = tmp6[:].rearrange("p (m k w) -> p m k w", m=2, k=3)
                for dstv, av in ((nav, ea5), (mmv, d35)):
                    a5 = av[:].rearrange("p (m k w) -> p m k w", m=2, k=5)
                    b5 = eb5[:].rearrange("p (m k w) -> p m k w", m=2, k=5)
                    nc.vector.tensor_tensor(
                        out=dstv, in0=a5[:, :, 1:4, :], in1=b5[:, :, 2:5, :],
                        op=Alu.mult,
                    )
                    nc.vector.tensor_tensor(
                        out=t6v, in0=a5[:, :, 2:5, :], in1=b5[:, :, 1:4, :],
                        op=Alu.mult,
                    )
                    nc.vector.tensor_tensor(
                        out=dstv, in0=dstv, in1=t6v, op=Alu.subtract
                    )

                # pq = [na*mm products | squares of na,mm]: the plane sums
                # for q and the norms then merge into one add pair in the tail
                pq = pip.tile([P, 18 * F], f16, tag="pq")
                nc.vector.tensor_tensor(
                    out=pq[:, 0 : 6 * F], in0=namm[:, 0 : 6 * F],
                    in1=namm[:, 6 * F : 12 * F], op=Alu.mult,
                )
                # squares on ACT (runs while DVE streams the next chunk)
                nc.scalar.activation(
                    out=pq[:, 6 * F : 18 * F], in_=namm[:], func=AF.Square
                )
                return pq

            def tail_stages(t, pq):
                """Chunk tail as 5 stages (each: DVE ops + the ACT hop they
                feed), emitted after the NEXT chunk's geometry; the last two
                chunks' stages are interleaved so the final ACT round-trips
                overlap DVE work from the sibling chunk."""
                st = {}

                def s1():
                    # plane sums: [q | na2 | m2] for both meshes in one pair
                    pqv = pq[:].rearrange("p (g k w) -> p g k w", g=6, k=3)
                    qnm = smp.tile([P, 6 * F], f16, tag="qnm")
                    qv = qnm[:].rearrange("p (g w) -> p g w", g=6)
                    nc.vector.tensor_tensor(
                        out=qv, in0=pqv[:, :, 0, :], in1=pqv[:, :, 1, :], op=Alu.add
                    )
                    nc.vector.tensor_tensor(
                        out=qv, in0=qv, in1=pqv[:, :, 2, :], op=Alu.add
                    )
                    st["qnm"] = qnm

                def s2():
                    # w = sqrt(na2 * m2 + eps); eps keeps padded/degenerate
                    # edges finite (q = 0 there -> t = 1 -> delta 0)
                    qnm = st["qnm"]
                    pr2 = smp.tile([P, 2 * F], f32, tag="pr2")
                    nc.vector.tensor_tensor(
                        out=pr2[:], in0=qnm[:, 2 * F : 4 * F],
                        in1=qnm[:, 4 * F : 6 * F], op=Alu.mult,
                    )
                    w01 = smp.tile([P, 2 * F], f32, tag="w01")
                    nc.scalar.activation(out=w01[:], in_=pr2[:], func=AF.Sqrt, bias=EPS)
                    st["w01"] = w01

                def s3():
                    # t = sqrt((w-q)/(w+q)) = exp(0.5*(ln(w-q) - ln(w+q))).
                    # aa|bb in one [P, 4F] tile: one clamp, one Ln.  Ln bias
                    # bounds t for the measure-zero edges the reference clips;
                    # max(0) guards f32 rounding pushing w -+ q negative.
                    qnm, w01 = st["qnm"], st["w01"]
                    q01 = qnm[:, 0 : 2 * F]
                    ab = smp.tile([P, 4 * F], f32, tag="ab")
                    nc.vector.tensor_tensor(
                        out=ab[:, 0 : 2 * F], in0=w01[:], in1=q01, op=Alu.subtract
                    )
                    nc.vector.tensor_tensor(
                        out=ab[:, 2 * F : 4 * F], in0=w01[:], in1=q01, op=Alu.add
                    )
                    nc.vector.tensor_scalar(
                        out=ab[:], in0=ab[:], scalar1=0.0, scalar2=None, op0=Alu.max
                    )
                    nc.scalar.activation(out=ab[:], in_=ab[:], func=AF.Ln, bias=EPS2)
                    st["lab"] = ab

                def s4():
                    lab = st["lab"]
                    zv = smp.tile([P, 2 * F], f32, tag="zv")
                    nc.vector.tensor_tensor(
                        out=zv[:], in0=lab[:, 0 : 2 * F], in1=lab[:, 2 * F : 4 * F],
                        op=Alu.subtract,
                    )
                    nc.scalar.activation(out=zv[:], in_=zv[:], func=AF.Exp, scale=0.5)
                    nc.scalar.activation(out=zv[:], in_=zv[:], func=AF.Arctan)
                    st["at"] = zv

                def s5():
                    # delta = at0 - at1; acc[:, t] = sum delta^2 via ACT accum
                    at = st["at"]
                    d = smp.tile([P, F], f32, tag="d")
                    nc.vector.tensor_tensor(
                        out=d[:], in0=at[:, 0:F], in1=at[:, F : 2 * F],
                        op=Alu.subtract,
                    )
                    dd = smp.tile([P, F], f32, tag="dd")
                    nc.scalar.activation(
                        out=dd[:], in_=d[:], func=AF.Square,
                        accum_out=acc[:, t : t + 1],
                    )

                return [s1, s2, s3, s4, s5]

            pend = None
            for t in range(T):
                pq = geo(t)
                if pend is not None and t < T - 1:
                    for s in tail_stages(*pend):
                        s()
                elif pend is not None:
                    # interleave the last two tails stage-by-stage
                    st_a = tail_stages(*pend)
                    st_b = tail_stages(t, pq)
                    for sa, sb in zip(st_a, st_b):
                        sa()
                        sb()
                    pq = None
                pend = (t, pq)

            nc.sync.dma_start(out=out[:, :], in_=acc[:])

    _split_multi_waits(nc)
    return nc


def _split_multi_waits(nc: bass.Bass) -> None:
    """Walrus accepts at most ONE sync wait per (non-drain) instruction;
    hoist extras onto injected same-engine event-semaphore instructions
    placed immediately before -- semantically identical."""
    import bass_rust

    ctr = 0
    for fn in nc.m.functions:
        for bb in fn.blocks:
            new_list = []
            for inst in bb.instructions:
                si = getattr(inst, "sync_info", None)
                if si is not None and len(si.on_wait) > 1:
                    waits = list(si.on_wait)
                    for w in waits[:-1]:
                        ev = mybir.InstEventSemaphore(name=f"I-waitsplit-{ctr}")
                        ctr += 1
                        ev.engine = inst.engine
                        ev.sync_info = bass_rust.SyncInfo(
                            on_wait=[w], on_update=[]
                        )
                        new_list.append(ev)
                    inst.sync_info = bass_rust.SyncInfo(
                        on_wait=[waits[-1]], on_update=list(si.on_update)
                    )
                new_list.append(inst)
            bb.instructions = new_list


def _get_nc() -> bass.Bass:
    if "nc" not in _CACHE:
        _CACHE["nc"] = _build_program()
    return _CACHE["nc"]


def _prep_in_maps(vert1, vert2, edge_points):
    in_maps = []
    for b in range(B):
        tbl = np.concatenate(
            [np.asarray(vert1[b], np.float32), np.asarray(vert2[b], np.float32)],
            axis=1,
        )  # [N, 6]
        ep = np.asarray(edge_points[b]).astype(np.int32)  # [E, 4]
        pad = np.zeros((EPAD, 4), np.int32)
        pad[:E] = ep
        # edge (t, p, w) = (t*P + p)*F + w; gather rows then lay out
        # plane-major: pg[t, p, (c, j, w)] with c = 3*mesh + xyz
        g = tbl.astype(np.float16)[pad.reshape(T, P, F, 4)]  # [T, P, F, 4, 6]
        pgb = np.ascontiguousarray(g.transpose(0, 1, 4, 3, 2)).reshape(T, P, 24 * F)
        in_maps.append({"pg": pgb})
    return in_maps


def _run(in_maps, **kwargs):
    nc = _get_nc()
    return run_bass_kernel_spmd(nc, in_maps, core_ids=list(range(B)), **kwargs)


def _finalize(results) -> np.float32:
    total = 0.0
    for rmap in results:
        total += float(np.asarray(rmap["out"], np.float64).sum())
    # angle diff = 2*(atan1 - atan2)  ->  factor 4 on the squared sums
    return np.float32(4.0 * total / (B * E))


def kernel(vert1, vert2, edge_points) -> np.ndarray:
    in_maps = _prep_in_maps(vert1, vert2, edge_points)
    res = _run(in_maps)
    return _finalize(res.results)


# revision 18
# speedup vs baseline: 1.8969x; 1.0564x over previous
"""Trainium2 Bass kernel for MeshDihedralAngleLoss.

Reference computation (per batch b, per edge e with ep = edge_points[b,e] =
[v0, v1, a, b]):
    na = normalize(cross(verts[a]-verts[v0], verts[v1]-verts[v0]))
    nb = normalize(cross(verts[b]-verts[v1], verts[v0]-verts[v1]))
    angle = pi - arccos(clip(dot(na, nb), +-(1-1e-7)))
computed for vert1 and vert2; loss = mean_b mean_e (angle1-angle2)^2.

Algebra: with ea = p2-p0, eb = p1-p0, d3 = p3-p1, nb = cross(d3, -eb) = -m,
    angle = pi - arccos(-u) = arccos(u),  u = dot(na, m)/(|na||m|)
and with q = dot(na, m), w = |na||m|:
    arccos(u) = 2*atan(sqrt((w - q)/(w + q)))
              = 2*atan(exp(0.5*(ln(w - q) - ln(w + q))))     [division-free]
so angle1 - angle2 = 2*(atan(t1) - atan(t2)) and the host applies the *4
factor on the squared sums plus the global mean (the only cross-core step).

Sharding: pure data parallel, core b <- mesh b (B == 8 == n_cores).

Host marshaling: the indexed gather is pure data movement, so it is done
host-side with numpy fancy indexing (same class of marshaling as index
pre-tiling): each core receives its edges' vertex coordinates pre-gathered
(fp16) into the exact plane-major SBUF layout
    pg[t, p, ((m*3 + c)*4 + j)*F + w] = verts_m[ep[e, j], c],
    e = (t*P + p)*F + w
(m = mesh 0/1, c = xyz, j = vertex slot 0..3).  The device then streams
sequential DRAM at full DMA bandwidth -- no per-edge descriptors.

Engine split (software-pipelined by one chunk so DVE never waits on ACT):
  DVE:  edge vectors, cross products, dot products (fp16, 2x mode) + f32 glue
  ACT:  squares, sqrt, ln/exp (the division), arctan, final sum-accumulate
"""

import numpy as np

import concourse.bass as bass
import concourse.mybir as mybir
from concourse.tile import TileContext
from concourse.bass_utils import run_bass_kernel_spmd

dt = mybir.dt
Alu = mybir.AluOpType
AF = mybir.ActivationFunctionType

B, N, E = 8, 100000, 300000
P = 128
F = 586            # edges per partition per chunk
T = 4              # chunks; P*F*T = 300032 >= E (32 zero-padded edges)
EPAD = P * F * T
EPS = 1e-30
EPS2 = 1e-35

_CACHE: dict = {}


def _build_program() -> bass.Bass:
    nc = bass.Bass(trn_type="TRN2")
    f32 = dt.float32
    f16 = dt.float16
    # register the eps consts used as ACT biases (same mechanism as the
    # 0.0/1.0 consts Bass registers at init)
    for cv in (EPS, EPS2):
        eps_t = nc.alloc_sbuf_tensor(f"const-float32-{cv}", [128, 1], f32)
        nc.gpsimd.memset(eps_t.ap(), cv)
        nc.const_aps.aps[(f32, cv)] = eps_t.ap()
    nc.all_engine_barrier()
    pg = nc.dram_tensor("pg", [T, P, 24 * F], f16, kind="ExternalInput")
    out = nc.dram_tensor("out", [P, T], f32, kind="ExternalOutput")

    with TileContext(nc) as tc:
        with (
            tc.tile_pool(name="accp", bufs=1) as accp,
            tc.tile_pool(name="iop", bufs=2) as iop,
            tc.tile_pool(name="wkp", bufs=1) as wkp,
            tc.tile_pool(name="pip", bufs=2) as pip,
            tc.tile_pool(name="smp", bufs=1) as smp,
        ):
            acc = accp.tile([P, T], f32)

            def geo(t):
                """DVE geometry for chunk t + ACT squares; returns (q01, namm)."""
                gall = iop.tile([P, 24 * F], f16, tag="gall")
                nc.sync.dma_start(out=gall[:], in_=pg[t, :, :])
                # planes: [m:2][c:3][j:4][w:F]
                gv = gall[:].rearrange("p (m c j w) -> p m c j w", m=2, c=3, j=4)

                # Edge vectors, both meshes at once, into 5-plane buffers
                # (x,y,z,x,y per mesh) so cross-product component rotation
                # becomes a plain +F / +2F offset shift.
                ea5 = wkp.tile([P, 10 * F], f16, tag="ea5")
                eb5 = wkp.tile([P, 10 * F], f16, tag="eb5")
                d35 = wkp.tile([P, 10 * F], f16, tag="d35")
                for buf, js, jb in ((ea5, 2, 0), (eb5, 1, 0), (d35, 3, 1)):
                    bv = buf[:].rearrange("p (m k w) -> p m k w", m=2, k=5)
                    nc.vector.tensor_tensor(
                        out=bv[:, :, 0:3, :],
                        in0=gv[:, :, :, js, :],
                        in1=gv[:, :, :, jb, :],
                        op=Alu.subtract,
                    )
                    nc.vector.tensor_tensor(
                        out=bv[:, :, 3:5, :],
                        in0=gv[:, :, 0:2, js, :],
                        in1=gv[:, :, 0:2, jb, :],
                        op=Alu.subtract,
                    )

                # na = ea x eb ; mm = d3 x eb  (plane-major, per mesh)
                # packed as namm = [na_m0 | na_m1 | mm_m0 | mm_m1] x 3F
                namm = pip.tile([P, 12 * F], f16, tag="namm")
                nav = namm[:, 0 : 6 * F].rearrange("p (m k w) -> p m k w", m=2, k=3)
                mmv = namm[:, 6 * F : 12 * F].rearrange("p (m k w) -> p m k w", m=2, k=3)
                tmp6 = wkp.tile([P, 6 * F], f16, tag="t6")
                t6v = tmp6[:].rearrange("p (m k w) -> p m k w", m=2, k=3)
                for dstv, av in ((nav, ea5), (mmv, d35)):
                    a5 = av[:].rearrange("p (m k w) -> p m k w", m=2, k=5)
                    b5 = eb5[:].rearrange("p (m k w) -> p m k w", m=2, k=5)
                    nc.vector.tensor_tensor(
                        out=dstv, in0=a5[:, :, 1:4, :], in1=b5[:, :, 2:5, :],
                        op=Alu.mult,
                    )
                    nc.vector.tensor_tensor(
                        out=t6v, in0=a5[:, :, 2:5, :], in1=b5[:, :, 1:4, :],
                        op=Alu.mult,
                    )
                    nc.vector.tensor_tensor(
                        out=dstv, in0=dstv, in1=t6v, op=Alu.subtract
                    )

                # pq = [na*mm products | squares of na,mm]: the plane sums
                # for q and the norms then merge into one add pair in the tail
                pq = pip.tile([P, 18 * F], f16, tag="pq")
                nc.vector.tensor_tensor(
                    out=pq[:, 0 : 6 * F], in0=namm[:, 0 : 6 * F],
                    in1=namm[:, 6 * F : 12 * F], op=Alu.mult,
                )
                # squares on ACT (runs while DVE streams the next chunk)
                nc.scalar.activation(
                    out=pq[:, 6 * F : 18 * F], in_=namm[:], func=AF.Square
                )
                return pq

            def tail_stages(t, pq):
                """Chunk tail as 5 stages (each: DVE ops + the ACT hop they
                feed), emitted after the NEXT chunk's geometry; the last two
                chunks' stages are interleaved so the final ACT round-trips
                overlap DVE work from the sibling chunk."""
                st = {}

                def s1():
                    # plane sums: [q | na2 | m2] for both meshes in one pair
                    pqv = pq[:].rearrange("p (g k w) -> p g k w", g=6, k=3)
                    qnm = smp.tile([P, 6 * F], f16, tag="qnm")
                    qv = qnm[:].rearrange("p (g w) -> p g w", g=6)
                    nc.vector.tensor_tensor(
                        out=qv, in0=pqv[:, :, 0, :], in1=pqv[:, :, 1, :], op=Alu.add
                    )
                    nc.vector.tensor_tensor(
                        out=qv, in0=qv, in1=pqv[:, :, 2, :], op=Alu.add
                    )
                    st["qnm"] = qnm

                def s2():
                    # w = |na||m| = sqrt(na2)*sqrt(m2), all fp16 (2x DVE).
                    # Degenerate/padded edges: w = q = 0 -> ln(eps)-ln(eps)
                    # = 0 -> t = 1 -> both meshes pi/4 -> delta 0.
                    qnm = st["qnm"]
                    sn = smp.tile([P, 4 * F], f16, tag="sn")
                    nc.scalar.activation(
                        out=sn[:], in_=qnm[:, 2 * F : 6 * F], func=AF.Sqrt
                    )
                    w01 = smp.tile([P, 2 * F], f16, tag="w01")
                    nc.vector.tensor_tensor(
                        out=w01[:], in0=sn[:, 0 : 2 * F], in1=sn[:, 2 * F : 4 * F],
                        op=Alu.mult,
                    )
                    st["w01"] = w01

                def s3():
                    # t = sqrt((w-q)/(w+q)) = exp(0.5*(ln(w-q) - ln(w+q))).
                    # aa|bb in one [P, 4F] tile: one clamp, one Ln.  Ln bias
                    # bounds t for the measure-zero edges the reference clips;
                    # max(0) guards f32 rounding pushing w -+ q negative.
                    qnm, w01 = st["qnm"], st["w01"]
                    q01 = qnm[:, 0 : 2 * F]
                    ab = smp.tile([P, 4 * F], f16, tag="ab")
                    nc.vector.tensor_tensor(
                        out=ab[:, 0 : 2 * F], in0=w01[:], in1=q01, op=Alu.subtract
                    )
                    nc.vector.tensor_tensor(
                        out=ab[:, 2 * F : 4 * F], in0=w01[:], in1=q01, op=Alu.add
                    )
                    nc.vector.tensor_scalar(
                        out=ab[:], in0=ab[:], scalar1=0.0, scalar2=None, op0=Alu.max
                    )
                    nc.scalar.activation(out=ab[:], in_=ab[:], func=AF.Ln, bias=EPS2)
                    st["lab"] = ab

                def s4():
                    lab = st["lab"]
                    zv = smp.tile([P, 2 * F], f16, tag="zv")
                    nc.vector.tensor_tensor(
                        out=zv[:], in0=lab[:, 0 : 2 * F], in1=lab[:, 2 * F : 4 * F],
                        op=Alu.subtract,
                    )
                    # Exp out in f32: t can overflow fp16 for near-pi angles
                    tv = smp.tile([P, 2 * F], f32, tag="tv")
                    nc.scalar.activation(out=tv[:], in_=zv[:], func=AF.Exp, scale=0.5)
                    nc.scalar.activation(out=tv[:], in_=tv[:], func=AF.Arctan)
                    st["at"] = tv

                def s5():
                    # delta = at0 - at1; acc[:, t] = sum delta^2 via ACT accum
                    at = st["at"]
                    d = smp.tile([P, F], f32, tag="d")
                    nc.vector.tensor_tensor(
                        out=d[:], in0=at[:, 0:F], in1=at[:, F : 2 * F],
                        op=Alu.subtract,
                    )
                    dd = smp.tile([P, F], f32, tag="dd")
                    nc.scalar.activation(
                        out=dd[:], in_=d[:], func=AF.Square,
                        accum_out=acc[:, t : t + 1],
                    )

                return [s1, s2, s3, s4, s5]

            pend = None
            for t in range(T):
                pq = geo(t)
                if pend is not None and t < T - 1:
                    for s in tail_stages(*pend):
                        s()
                elif pend is not None:
                    # interleave the last two tails stage-by-stage
                    st_a = tail_stages(*pend)
                    st_b = tail_stages(t, pq)
                    for sa, sb in zip(st_a, st_b):
                        sa()
                        sb()
                    pq = None
                pend = (t, pq)

            nc.sync.dma_start(out=out[:, :], in_=acc[:])

    _split_multi_waits(nc)
    return nc


def _split_multi_waits(nc: bass.Bass) -> None:
    """Two post-scheduling wait cleanups:

    1. Drop redundant waits: each engine's sequencer executes waits in
       program order and semaphore values are monotone within the kernel
       body, so a wait on (sem >= v) is a no-op if an earlier instruction
       on the same engine already waited (sem >= v') with v' >= v.  Dedup
       stops at the first DRAIN (the kernel-tail drain resets sems).
    2. Walrus accepts at most ONE sync wait per (non-drain) instruction;
       hoist extras onto injected same-engine event-semaphore instructions
       placed immediately before -- semantically identical."""
    import bass_rust

    ctr = 0
    for fn in nc.m.functions:
        for bb in fn.blocks:
            observed: dict = {}  # (engine, sem_id) -> max waited value
            dedup_on = True
            new_list = []
            for inst in bb.instructions:
                if isinstance(inst, mybir.InstDrain):
                    dedup_on = False
                si = getattr(inst, "sync_info", None)
                if si is not None and si.on_wait and dedup_on:
                    kept = []
                    for w in si.on_wait:
                        if (
                            w.sync_type == "semaphore"
                            and w.wait_mode == "sem-ge-imm"
                            and getattr(w, "wait_reg", None) is None
                        ):
                            key = (str(inst.engine), w.id)
                            if observed.get(key, -1) >= w.wait_value:
                                continue
                            observed[key] = w.wait_value
                        kept.append(w)
                    if len(kept) != len(si.on_wait):
                        si = bass_rust.SyncInfo(
                            on_wait=kept, on_update=list(si.on_update)
                        )
                        inst.sync_info = si
                if si is not None and len(si.on_wait) > 1:
                    waits = list(si.on_wait)
                    for w in waits[:-1]:
                        ev = mybir.InstEventSemaphore(name=f"I-waitsplit-{ctr}")
                        ctr += 1
                        ev.engine = inst.engine
                        ev.sync_info = bass_rust.SyncInfo(
                            on_wait=[w], on_update=[]
                        )
                        new_list.append(ev)
                    inst.sync_info = bass_rust.SyncInfo(
                        on_wait=[waits[-1]], on_update=list(si.on_update)
                    )
                new_list.append(inst)
            bb.instructions = new_list


def _get_nc() -> bass.Bass:
    if "nc" not in _CACHE:
        _CACHE["nc"] = _build_program()
    return _CACHE["nc"]


def _prep_in_maps(vert1, vert2, edge_points):
    in_maps = []
    for b in range(B):
        tbl = np.concatenate(
            [np.asarray(vert1[b], np.float32), np.asarray(vert2[b], np.float32)],
            axis=1,
        )  # [N, 6]
        ep = np.asarray(edge_points[b]).astype(np.int32)  # [E, 4]
        pad = np.zeros((EPAD, 4), np.int32)
        pad[:E] = ep
        # edge (t, p, w) = (t*P + p)*F + w; gather rows then lay out
        # plane-major: pg[t, p, (c, j, w)] with c = 3*mesh + xyz
        g = tbl.astype(np.float16)[pad.reshape(T, P, F, 4)]  # [T, P, F, 4, 6]
        pgb = np.ascontiguousarray(g.transpose(0, 1, 4, 3, 2)).reshape(T, P, 24 * F)
        in_maps.append({"pg": pgb})
    return in_maps


def _run(in_maps, **kwargs):
    nc = _get_nc()
    return run_bass_kernel_spmd(nc, in_maps, core_ids=list(range(B)), **kwargs)


def _finalize(results) -> np.float32:
    total = 0.0
    for rmap in results:
        total += float(np.asarray(rmap["out"], np.float64).sum())
    # angle diff = 2*(atan1 - atan2)  ->  factor 4 on the squared sums
    return np.float32(4.0 * total / (B * E))


def kernel(vert1, vert2, edge_points) -> np.ndarray:
    in_maps = _prep_in_maps(vert1, vert2, edge_points)
    res = _run(in_maps)
    return _finalize(res.results)
